# revision 1
# baseline (speedup 1.0000x reference)
"""Child-Sum Tree-LSTM cell on 8 Trainium2 NeuronCores (Bass/Tile).

Data-parallel over the batch axis: each core gets B/8 = 4096 rows of
x/h/C plus replicated [128,128] weights, computes (h_j, c_j) for its
shard, and the host concatenates the shards.

Per-core kernel, processed in macro-tiles of `mt` 128-row tiles (rows
are interleaved row = b*mt + m so the h/C DMAs stay 3-dim and >=2MB):
  - DMA-cast x/h/C fp32->fp16 into SBUF (SWDGE casts inline).
  - PE transposes x and each h_n (fp16, via identity) so they can serve
    as matmul stationaries; ScalarE evacuates them to SBUF.
  - h_tilde^T per sub-tile via a batched VectorE tree over the h_n^T.
  - Gate pre-activations assembled entirely in PSUM accumulation:
      A = x@[Wi|Wo|Wu] + 1(x)[bi|bo|bu] + h_tilde@[Ui|Uo|Uu]
      F_n = x@Wf + 1(x)bf + h_n@Uf      (all 8 children)
    (biases enter as rank-1 K=1 matmuls with a ones stationary)
  - ScalarE applies sigmoid/tanh straight out of PSUM (fp16 out).
  - VectorE does the f (.) C multiply + child-tree reduction and the
    final c = i*u + fc, h = o*tanh(c), batched across the whole
    macro-tile to amortize per-op overhead (fp32 outputs).
"""

import numpy as np

D = 128
NCH = 8
NCORES = 8
BATCH = 32768
P = 128

_CACHE = {}


def build_nc(b_loc, variant="full"):
    import os as _os
    import re as _re
    from contextlib import ExitStack

    import concourse.tile as tile
    from concourse import bacc, mybir
    from concourse.masks import make_identity

    f32 = mybir.dt.float32
    f16 = mybir.dt.float16

    ntiles = b_loc // P
    assert b_loc % P == 0

    reps = int(_os.environ.get("KV_REPS", "1"))
    mt = int(_os.environ.get("KV_MT", "2"))
    if variant not in ("full", "dma_only", "compute_only"):
        mm = _re.fullmatch(r"(?:mt(\d+))?(?:rep(\d+))?", variant)
        assert mm, f"bad variant {variant}"
        if mm.group(1):
            mt = int(mm.group(1))
        if mm.group(2):
            reps = int(mm.group(2))
        variant = "full"

    nc = bacc.Bacc("TRN2", target_bir_lowering=False, debug=False)

    x_d = nc.dram_tensor("x", [b_loc, D], f32, kind="ExternalInput")
    h_d = nc.dram_tensor("h", [NCH, b_loc, D], f32, kind="ExternalInput")
    C_d = nc.dram_tensor("C", [NCH, b_loc, D], f32, kind="ExternalInput")
    Wd = {
        n: nc.dram_tensor(n, [D, D], f32, kind="ExternalInput")
        for n in ("W_i", "W_f", "W_o", "W_u", "U_i", "U_f", "U_o", "U_u")
    }
    bd = {
        n: nc.dram_tensor(n, [1, D], f32, kind="ExternalInput")
        for n in ("b_i", "b_f", "b_o", "b_u")
    }
    h_o = nc.dram_tensor("h_out", [b_loc, D], f32, kind="ExternalOutput")
    c_o = nc.dram_tensor("c_out", [b_loc, D], f32, kind="ExternalOutput")

    with ExitStack() as ctx:
        tc = ctx.enter_context(tile.TileContext(nc))
        wbufs = int(_os.environ.get("KV_WBUFS", "2"))
        lbufs = int(_os.environ.get("KV_LBUFS", "5"))
        consts = ctx.enter_context(tc.tile_pool(name="consts", bufs=1))
        loads = ctx.enter_context(tc.tile_pool(name="loads", bufs=lbufs))
        work = ctx.enter_context(tc.tile_pool(name="work", bufs=wbufs))
        outp = ctx.enter_context(tc.tile_pool(name="outp", bufs=wbufs))
        # PSUM budget (8 banks): tp 2x2 + A 1x2 + F 2x1 = 8
        tp_ps = ctx.enter_context(tc.tile_pool(name="tp_ps", bufs=2, space="PSUM"))
        a_ps = ctx.enter_context(tc.tile_pool(name="a_ps", bufs=2, space="PSUM"))
        f_ps = ctx.enter_context(tc.tile_pool(name="f_ps", bufs=1, space="PSUM"))

        # ---- one-time constants -------------------------------------------
        ident = consts.tile([P, P], f16)
        make_identity(nc, ident)
        ones = consts.tile([1, P], f16)
        nc.vector.memset(ones, 1.0)

        Wcat = consts.tile([P, 3, D], f16)  # [Wi|Wo|Wu]
        Ucat = consts.tile([P, 3, D], f16)  # [Ui|Uo|Uu]
        bcat = consts.tile([1, 3, D], f16)  # [bi|bo|bu]
        for j, (w, u, b) in enumerate(
            (("W_i", "U_i", "b_i"), ("W_o", "U_o", "b_o"), ("W_u", "U_u", "b_u"))
        ):
            nc.gpsimd.dma_start(Wcat[:, j, :], Wd[w][:, :])
            nc.gpsimd.dma_start(Ucat[:, j, :], Wd[u][:, :])
            nc.gpsimd.dma_start(bcat[:, j, :], bd[b][:, :])
        Uf = consts.tile([P, D], f16)
        nc.gpsimd.dma_start(Uf, Wd["U_f"][:, :])
        Wf4 = consts.tile([P, 4, D], f16)  # W_f replicated 4x (one PSUM bank wide)
        bf4 = consts.tile([1, 4, D], f16)
        for j in range(4):
            nc.gpsimd.dma_start(Wf4[:, j, :], Wd["W_f"][:, :])
            nc.gpsimd.dma_start(bf4[:, j, :], bd["b_f"][:, :])

        if variant == "dma_only":
            zc = consts.tile([P, D], f32)
            nc.vector.memset(zc, 0.0)
            zh = consts.tile([P, D], f32)
            nc.vector.memset(zh, 0.0)

        if variant == "compute_only":
            x_sb0 = consts.tile([P, mt, D], f16)
            nc.gpsimd.dma_start(
                x_sb0, x_d[0 : mt * P, :].rearrange("(b m) k -> b (m k)", b=P)
            )
            h_sb0 = consts.tile([P, NCH, mt, D], f16)
            nc.gpsimd.dma_start(
                h_sb0, h_d[:, 0 : mt * P, :].rearrange("n (b m) k -> b n (m k)", b=P)
            )
            C_sb0 = consts.tile([P, NCH, mt, D], f16)
            nc.gpsimd.dma_start(
                C_sb0, C_d[:, 0 : mt * P, :].rearrange("n (b m) k -> b n (m k)", b=P)
            )

        Sig = mybir.ActivationFunctionType.Sigmoid
        Tanh = mybir.ActivationFunctionType.Tanh

        # ---- main loop over macro-tiles (mt row-tiles each) ---------------
        assert ntiles % mt == 0
        for m in range(ntiles * reps // mt):
            m = m % (ntiles // mt)
            r0 = m * mt * P

            if variant == "compute_only":
                x_mt, h_mt, C_mt = x_sb0, h_sb0, C_sb0
            elif variant == "dma_only" and _os.environ.get("KV_LOADENG") == "sync":
                # diagnostic: plain fp32 loads through HWDGE (no SWDGE cast)
                x_mt = loads.tile([P, mt, D], f32, tag="x_sb")
                nc.sync.dma_start(
                    x_mt,
                    x_d[r0 : r0 + mt * P, :].rearrange("(b m) k -> b (m k)", b=P),
                )
                h_mt = loads.tile([P, NCH, mt, D], f32, tag="h_sb")
                nc.sync.dma_start(
                    h_mt,
                    h_d[:, r0 : r0 + mt * P, :].rearrange(
                        "n (b m) k -> b n (m k)", b=P
                    ),
                )
                C_mt = loads.tile([P, NCH, mt, D], f32, tag="C_sb")
                nc.scalar.dma_start(
                    C_mt,
                    C_d[:, r0 : r0 + mt * P, :].rearrange(
                        "n (b m) k -> b n (m k)", b=P
                    ),
                )
            else:
                # rows interleaved: DRAM row r0 + b*mt + m -> partition b,
                # sub-tile m; (m k) stays contiguous so DMA APs are 3-dim.
                x_mt = loads.tile([P, mt, D], f16, tag="x_sb")
                nc.gpsimd.dma_start(
                    x_mt,
                    x_d[r0 : r0 + mt * P, :].rearrange("(b m) k -> b (m k)", b=P),
                )
                h_mt = loads.tile([P, NCH, mt, D], f16, tag="h_sb")
                nc.gpsimd.dma_start(
                    h_mt,
                    h_d[:, r0 : r0 + mt * P, :].rearrange(
                        "n (b m) k -> b n (m k)", b=P
                    ),
                )
                C_mt = loads.tile([P, NCH, mt, D], f16, tag="C_sb")
                nc.gpsimd.dma_start(
                    C_mt,
                    C_d[:, r0 : r0 + mt * P, :].rearrange(
                        "n (b m) k -> b n (m k)", b=P
                    ),
                )

            if variant == "dma_only":
                # touch the loaded tiles so DCE keeps the DMAs
                dmy = work.tile([P, 1], f32, tag="dmy")
                nc.vector.tensor_add(dmy, h_mt[:, 0, 0, 0:1], C_mt[:, 0, 0, 0:1])
                nc.vector.tensor_add(dmy, dmy, x_mt[:, 0, 0:1])
                for s in range(mt):
                    nc.sync.dma_start(c_o[r0 + s * P : r0 + (s + 1) * P, :], zc)
                    nc.sync.dma_start(h_o[r0 + s * P : r0 + (s + 1) * P, :], zh)
                continue

            c_mt = outp.tile([P, mt, D], f32, tag="c_mt")
            hh_mt = outp.tile([P, mt, D], f32, tag="hh_mt")
            tps = work.tile([P, mt, 9, D], f16, tag="tps")
            f_all = work.tile([P, NCH, mt, D], f16, tag="f_all")
            io_all = work.tile([P, 2, mt, D], f16, tag="io_all")
            u_all = work.tile([P, mt, D], f16, tag="u_all")

            # Phase 1: per-sub-tile transposes (PE) + evacuation (ScalarE).
            for s in range(mt):
                tp = tp_ps.tile([P, 9, D], f16, tag="tp")
                for n in range(NCH):
                    nc.tensor.matmul(
                        tp[:, n, :],
                        h_mt[:, n, s, :],
                        ident,
                        is_transpose=True,
                        start=(n == 0),
                        stop=(n == NCH - 1),
                    )
                nc.tensor.matmul(
                    tp[:, 8, :],
                    x_mt[:, s, :],
                    ident,
                    is_transpose=True,
                    start=True,
                    stop=True,
                )
                if _os.environ.get("KV_TPSCOPY", "act") == "dve":
                    nc.vector.tensor_copy(tps[:, s, :, :], tp)
                else:
                    nc.scalar.copy(tps[:, s, :, :], tp)

            # Phase 2: h_tilde^T = sum_n h_n^T, batched tree on VectorE.
            s1h = work.tile([P, mt, 4, D], f16, tag="s1h")
            nc.vector.tensor_add(s1h, tps[:, :, 0:4, :], tps[:, :, 4:8, :])
            s2h = work.tile([P, mt, 2, D], f16, tag="s2h")
            nc.vector.tensor_add(s2h, s1h[:, :, 0:2, :], s1h[:, :, 2:4, :])
            hsT = work.tile([P, mt, D], f16, tag="hsT")
            nc.vector.tensor_add(hsT, s2h[:, :, 0, :], s2h[:, :, 1, :])

            # Phase 3: gate pre-activations in PSUM + activations.
            for s in range(mt):
                xT = tps[:, s, 8, :]
                A = a_ps.tile([P, 3, D], f32, tag="A")
                nc.tensor.matmul(A, xT, Wcat, start=True, stop=False)
                nc.tensor.matmul(A, ones, bcat, start=False, stop=False)
                nc.tensor.matmul(A, hsT[:, s, :], Ucat, start=False, stop=True)

                F = f_ps.tile([P, NCH, D], f32, tag="F")
                for j in range(2):
                    blk = F[:, 4 * j : 4 * j + 4, :]
                    nc.tensor.matmul(blk, xT, Wf4, start=True, stop=False)
                    nc.tensor.matmul(blk, ones, bf4, start=False, stop=False)
                    for c in range(4):
                        n = 4 * j + c
                        nc.tensor.matmul(
                            F[:, n, :],
                            tps[:, s, n, :],
                            Uf,
                            start=False,
                            stop=(c == 3),
                        )

                nc.scalar.activation(io_all[:, :, s, :], A[:, 0:2, :], Sig)
                nc.scalar.activation(u_all[:, s, :], A[:, 2, :], Tanh)
                nc.scalar.activation(f_all[:, :, s, :], F, Sig)

            # Phase 4: batched elementwise over the whole macro-tile.
            tree_dt = f32 if _os.environ.get("KV_F32TREE") else f16
            prod = work.tile([P, NCH, mt, D], f16, tag="prod")
            nc.vector.tensor_mul(prod, f_all, C_mt)
            p1 = work.tile([P, 4, mt, D], tree_dt, tag="p1")
            nc.vector.tensor_add(p1, prod[:, 0:4, :, :], prod[:, 4:8, :, :])
            p2 = work.tile([P, 2, mt, D], tree_dt, tag="p2")
            nc.vector.tensor_add(p2, p1[:, 0:2, :, :], p1[:, 2:4, :, :])
            fc = work.tile([P, mt, D], tree_dt, tag="fc")
            nc.vector.tensor_add(fc, p2[:, 0, :, :], p2[:, 1, :, :])

            iu = work.tile([P, mt, D], f16, tag="iu")
            nc.vector.tensor_mul(iu, io_all[:, 0, :, :], u_all)
            nc.vector.tensor_add(c_mt, iu, fc)
            t_all = work.tile([P, mt, D], f16, tag="t_all")
            nc.scalar.activation(t_all, c_mt, Tanh)
            nc.vector.tensor_mul(hh_mt, io_all[:, 1, :, :], t_all)

            nc.sync.dma_start(
                c_o[r0 : r0 + mt * P, :].rearrange("(b m) k -> b (m k)", b=P), c_mt
            )
            nc.sync.dma_start(
                h_o[r0 : r0 + mt * P, :].rearrange("(b m) k -> b (m k)", b=P), hh_mt
            )

    nc.compile()
    return nc


def _shard_inputs(inputs, b_loc):
    n_shards = inputs["x"].shape[0] // b_loc
    in_maps = []
    for i in range(n_shards):
        s = slice(i * b_loc, (i + 1) * b_loc)
        m = {}
        for k, v in inputs.items():
            v = np.ascontiguousarray(np.asarray(v), dtype=np.float32)
            if k == "x":
                m[k] = np.ascontiguousarray(v[s])
            elif k in ("h", "C"):
                m[k] = np.ascontiguousarray(v[:, s])
            else:
                m[k] = v
        in_maps.append(m)
    return in_maps


def kernel(**inputs):
    from concourse.bass_utils import run_bass_kernel_spmd

    b_loc = BATCH // NCORES
    if b_loc not in _CACHE:
        _CACHE[b_loc] = build_nc(b_loc)
    nc = _CACHE[b_loc]

    in_maps = _shard_inputs(inputs, b_loc)
    res = run_bass_kernel_spmd(nc, in_maps, core_ids=list(range(NCORES)))
    h_full = np.concatenate([r["h_out"] for r in res.results], axis=0)
    c_full = np.concatenate([r["c_out"] for r in res.results], axis=0)
    return (h_full, c_full)

